# revision 120
# baseline (speedup 1.0000x reference)
"""Trainium2 Bass kernel for multi-head causal attention.

Problem: B=2, S=2048, D=1024, H=16, DH=64 (fp32), causal attention with
QKV projections and output projection summed over heads.

Sharding: 8 cores = (batch b in {0,1}) x (head-group hg in {0..3}, 4 heads
each).  Each core computes a partial output sum over its 4 heads for its
batch; the host sums the 4 partials per batch and adds b_O.

Key optimizations over the fp16 version:
  - Q/K projections run as fp8e4 DoubleRow matmuls (0.5 cyc/row, 2 d-tiles
    per instruction): x is quantized to e4m3 on the host, W_Q/W_K are shipped
    as hi+lo e4m3 pairs (hi = e4m3(W), lo = e4m3(W - hi)) so the weight
    quantization error cancels; two DoubleRow passes accumulate hi and lo.
  - Scores are fp8e4 DoubleRow: kT8 stores each 128-j-block padded with 128
    zero columns (the second DoubleRow k-tile), qT8 is read with a stride-0
    middle dim so one instruction computes k.T @ q at 0.5 cyc/col.  q is
    pre-scaled by 1/ATTN_SCALE before quantization.
  - V path and PV matmuls stay fp16 (p in fp8 costs ~1e-2 extra error).
  - The Activation engine does (almost) only exp; projection bias+scale run
    as DVE TensorScalar ops, the v ones-columns are preset once by a Pool
    memset, output is fp16.
  - Scheduling: all PE work that is not score/exp-critical (v projections,
    late kq chunks, deferred PVs, out-projections) flows through a global
    FIFO in dependency order and is pumped during strip gaps at a cycle
    budget matched to Act's exp pace, so Act (the bottleneck engine in the
    attention phase) stays saturated while the in-order PE stream never
    blocks at its head.  PVs trail scores/exp by a per-head lag, the first
    head's exps run per-512-chunk so Act starts as soon as kq chunk 0
    lands, and tail out-projection copies split across DVE and Act.

Measured end-to-end relative error vs the fp32 reference: 1.02e-2
(dominated by the e4m3 quantization of x and q/k; budget is 2e-2).
Cost-model makespan: 101,694 ns (baseline fp16 kernel: 145,547 ns).

A BIR post-processing patch (installed on import) hoists excess sync waits
off instructions into standalone EventSemaphore ops — walrus codegen allows
only 1 wait on the fused 4-byte-weight-load matmul encoding and few on
other opcodes, and Tile emits more.
"""

import sys

import numpy as np

for _p in ("/opt/trn_rl_repo",):
    if _p not in sys.path:
        sys.path.insert(0, _p)

import concourse.bass as bass
import concourse.tile as tile
from concourse import mybir
from concourse.bass_utils import run_bass_kernel_spmd


def _hoist_matmul_waits(bir_json: bytes) -> bytes:
    """Move extra sync waits off instructions into EventSemaphore ops.

    The fused 4-byte-weight-load matmul encoding only has room for one sync
    wait command in walrus codegen ("Too many sync wait commands").  Hoist
    all but one wait into standalone EventSemaphore instructions on the same
    engine queue immediately before the instruction — semantically identical
    (the sequencer blocks on them in order).
    """
    import orjson

    m = orjson.loads(bir_json)
    changed = False
    for fn in m.get("functions", []):
        for bb in fn.get("blocks", []):
            insts = bb.get("instructions", [])
            out = []
            for inst in insts:
                si = inst.get("sync_info") or {}
                waits = si.get("on_wait") or []
                if len(waits) > 1:
                    keep = waits[-1]
                    for wi, w in enumerate(waits[:-1]):
                        out.append({
                            "debug": inst.get("debug", 0),
                            "engine": inst["engine"],
                            "ins": [],
                            "name": f"{inst['name']}-hw{wi}",
                            "opcode": "EventSemaphore",
                            "outs": [],
                            "sync_info": {"on_update": [],
                                          "on_wait": [w]},
                        })
                    si["on_wait"] = [keep]
                    inst["sync_info"] = si
                    changed = True
                out.append(inst)
            bb["instructions"] = out
    if not changed:
        return bir_json
    return orjson.dumps(m)


def _install_bir_patch():
    from concourse import bass2jax as _b2j
    from concourse import bass_utils as _bu

    if getattr(_b2j, "_mm_wait_patch", False):
        return

    _orig = _bu.compile_bir_kernel

    def _patched(bir_json, tmpdir, neff_name="file.neff"):
        return _orig(_hoist_matmul_waits(bir_json), tmpdir, neff_name)

    _b2j.compile_bir_kernel = _patched
    _bu.compile_bir_kernel = _patched
    _b2j._mm_wait_patch = True


_install_bir_patch()

# Problem dims (hardcoded per harness contract).
B, S, D, H, DH = 2, 2048, 1024, 16, 64
ATTN_SCALE = 8.0
NCORES = 8
HL = H // (NCORES // B)  # 4 local heads per core
E = HL * DH              # 256 local head dims
P = 128
DC = D // P              # 8 contraction chunks of 128
DC2 = DC // 2            # 4 double-row d-pair chunks
EC = E // P              # 2 e-chunks
NSB = S // P             # 16 s-blocks of 128
NI = 1024                # i-group width for score strips
NG = S // NI             # 2 i-groups
F32 = mybir.dt.float32
F32R = mybir.dt.float32r
F16 = mybir.dt.float16
F8 = mybir.dt.float8e4
AF = mybir.ActivationFunctionType
ALU = mybir.AluOpType
PM = mybir.MatmulPerfMode


def _round_f32r(arr):
    """Round an fp32 array to float32r (tfloat32) representable values."""
    from neuronxcc.starfish.support import dtype as nxd
    a = np.ascontiguousarray(np.asarray(arr, dtype=np.float32))
    return np.asarray(nxd.static_cast(a, dtype=nxd.float32r)).view(np.float32)


def _stride0_pair(ap, n):
    """[K, N] AP -> [K, 2(stride 0), N] for the DoubleRow moving operand."""
    ap = ap.opt()
    dims = list(ap.ap)
    assert len(dims) == 2, dims
    return bass.AP(tensor=ap.tensor, offset=ap.offset,
                   ap=[dims[0], [0, 2], dims[1]])


def _emit(ctx, tc, ten):
    nc = tc.nc
    xq, xk, xv = ten["xq"], ten["xk"], ten["xv"]
    wo, bq, bk, bv, masks, out = (ten["wo"], ten["bq"], ten["bk"],
                                  ten["bv"], ten["masks"], ten["out"])

    persist = ctx.enter_context(tc.tile_pool(name="persist", bufs=1))
    xstage = ctx.enter_context(tc.tile_pool(name="xstage", bufs=8))
    xvstage = ctx.enter_context(tc.tile_pool(name="xvstage", bufs=8))
    ptpool = ctx.enter_context(tc.tile_pool(name="ptp", bufs=26))
    outpool = ctx.enter_context(tc.tile_pool(name="outp", bufs=4))
    small = ctx.enter_context(tc.tile_pool(name="small", bufs=6))
    # PSUM budget (8 banks of [128, 2KB]):
    #   ps_s: score strips [128, 1024] = 2 banks x 2 bufs = 4
    #   ps_mm: proj / outproj [128, <=512] = 1 bank x 2 bufs = 2
    #   ps_z: PV accumulators [128, 512] = 1 bank x 2 bufs = 2
    ps_s = ctx.enter_context(tc.tile_pool(name="ps_s", bufs=2, space="PSUM"))
    ps_mm = ctx.enter_context(tc.tile_pool(name="ps_mm", bufs=2, space="PSUM"))
    ps_z = ctx.enter_context(tc.tile_pool(name="ps_z", bufs=2, space="PSUM"))

    # --- persistent activations ---
    # qT8: [e-dims(128 = 2 heads), e-chunk, i-cols] fp8 (q pre-scaled by 1/8)
    qT8 = persist.tile([P, EC, S], F8)
    # kT8: per j-block 128 data cols + 128 zero cols (DoubleRow zero tile)
    kT8 = persist.tile([P, EC, NSB, 2 * P], F8)
    zT_sb = persist.tile([P, EC, S], F16)  # normalized z^T
    # v natural layout + 64 ones columns (rows 64..127 of PV psum become l)
    v_g = [persist.tile([P, NSB // NG, HL, 2 * DH], F16, name=f"v{g}")
           for g in range(NG)]

    xq_r = xq.rearrange("(c t p) s -> p c t s", p=P, t=2)
    xk_r = xk.rearrange("(c t p) s -> p c t s", p=P, t=2)
    xv_r = xv.rearrange("(c p) s -> p c s", p=P)

    w8_sb = {}
    for name in ("wqhi", "wqlo", "wkhi", "wklo"):
        w8_sb[name] = persist.tile([P, DC2, 2, E], F8, name=name)
    wv_sb = persist.tile([P, DC, E], F16)
    wo_sb = persist.tile([P, EC, D], F16)
    bq_sb = persist.tile([P, EC], F32)
    bk_sb = persist.tile([P, EC], F32)
    bv_bc = persist.tile([P, E], F32)
    masks_sb = persist.tile([P, P], F16)

    def emit_init():
        # zero the DoubleRow zero-tiles of kT8 and preset the v ones columns
        for m in range(EC):
            nc.gpsimd.memset(kT8[:, m, :, P:2 * P], 0.0)
        for g in range(NG):
            nc.gpsimd.memset(v_g[g][:, :, :, DH:2 * DH], 1.0)
        # warm up the Act engine's Exp table while DMAs stream in, so the
        # first real exp doesn't pay the ~1.3us table load
        warm = small.tile([1, 4], F32, tag="warm")
        nc.vector.memset(warm, 0.0)
        nc.scalar.activation(out=warm, in_=warm, func=AF.Exp)
        # ... and ramp the PE p-state (full clock needs ~3us of busy) with
        # throwaway matmuls while the first x chunks stream in
        wmm = small.tile([64, 512], F16, tag="wmm")
        nc.vector.memset(wmm, 0.0)
        wps = ps_mm.tile([64, 512], F32, tag="mm")
        for _ in range(7):
            nc.tensor.matmul(wps, lhsT=wmm[:, 0:64], rhs=wmm,
                             start=True, stop=True)

    staged = {}

    def load_kq_chunk(n, which):
        x_r = xk_r if which == "k" else xq_r
        xs = xstage.tile([P, DC2, 2, 512], F8, tag="xs")
        nc.sync.dma_start(out=xs, in_=x_r[:, :, :, n * 512:(n + 1) * 512])
        staged[(which, n)] = xs

    def load_kq_w(which):
        for nm in (("wkhi", "wklo") if which == "k" else ("wqhi", "wqlo")):
            nc.sync.dma_start(
                out=w8_sb[nm],
                in_=ten[nm].rearrange("p (c t e) -> p c t e", c=DC2, t=2))
        b_sb = bk_sb if which == "k" else bq_sb
        # small pre-packed bias loads issue from the idle Act queue so they
        # don't serialize the startup x/w stream on SP
        nc.scalar.dma_start(out=b_sb, in_=ten["bk" if which == "k" else "bq"])

    def load_v_piece(pc):
        """Load one 256-col (2-block) piece of xv."""
        if pc == 0:
            nc.sync.dma_start(out=wv_sb,
                              in_=ten["wv"].rearrange("(c p) e -> p c e", p=P))
            bv_bcast_ap = bass.AP(tensor=bv.tensor, offset=bv.offset,
                                  ap=[[0, P]] + list(bv.ap))
            nc.scalar.dma_start(out=bv_bc, in_=bv_bcast_ap)
        xs = xvstage.tile([P, DC, 256], F16, tag="xv")
        nc.sync.dma_start(out=xs, in_=xv_r[:, :, pc * 256:(pc + 1) * 256])
        staged[("v", pc)] = xs

    def emit_kq_part(n, which, m, on_act=False):
        """Project one e-chunk (m) of one 512-col chunk of k or q."""
        is_k = which == "k"
        whi = w8_sb["wkhi" if is_k else "wqhi"]
        wlo = w8_sb["wklo" if is_k else "wqlo"]
        b_sb = bk_sb if is_k else bq_sb
        scale = 1.0 if is_k else 1.0 / ATTN_SCALE
        xs = staged[(which, n)]
        ps = ps_mm.tile([P, 512], F32, tag="mm")
        idx = 0
        for dc2 in range(DC2):
            for w_sb in (whi, wlo):
                nc.tensor.matmul(
                    ps,
                    lhsT=w_sb[:, dc2, :, m * P:(m + 1) * P],
                    rhs=xs[:, dc2, :, :],
                    start=(idx == 0),
                    stop=(idx == 2 * DC2 - 1),
                    perf_mode=PM.DoubleRow,
                )
                idx += 1
        # dst = ps * scale + bias  (bias per-partition scalar)
        if is_k:
            out_ap = kT8[:, m, n * 4:(n + 1) * 4, 0:P]
            in_ap = ps.rearrange("p (b c) -> p b c", b=4)
        else:
            out_ap = qT8[:, m, n * 512:(n + 1) * 512]
            in_ap = ps[:]
        if on_act:
            # early chunks: Act is idle before the first exp, so the
            # psum->fp8 bias+scale runs there instead of on DVE
            nc.scalar.activation(
                out=out_ap, in_=in_ap, func=AF.Identity,
                bias=b_sb[:, m:m + 1], scale=scale)
        else:
            nc.vector.tensor_scalar(
                out=out_ap, in0=in_ap,
                scalar1=scale, scalar2=b_sb[:, m:m + 1],
                op0=ALU.mult, op1=ALU.add,
            )

    # v: staged in 256-col (2-block) pieces, projected per 128-block into v_g
    def emit_v_block(sb):
        xs = staged[("v", sb // 2)]
        sbl2 = sb % 2
        g, sbl = sb // (NSB // NG), sb % (NSB // NG)
        ps = ps_mm.tile([P, E], F32, tag="mm")
        for dc in range(DC):
            nc.tensor.matmul(
                ps,
                lhsT=xs[:, dc, sbl2 * P:(sbl2 + 1) * P],
                rhs=wv_sb[:, dc, :],
                start=(dc == 0),
                stop=(dc == DC - 1),
            )
        nc.vector.tensor_add(
            out=v_g[g][:, sbl, :, 0:DH],
            in0=ps.rearrange("p (h e) -> p h e", h=HL),
            in1=bv_bc.rearrange("p (h e) -> p h e", h=HL),
        )

    def attn_head(g, h, lag=2, sink=None):
        """Generator: emits one head of group g, yielding before each strip
        (and each drained PV) so the driver can interleave PE filler work.

        PV matmuls are emitted with a `lag`-strip lag behind scores/exp so
        the in-order PE stream never blocks on Act's exp.  With `sink`, the
        end-of-head PV drain is pushed into the global deferred-work FIFO
        (consumed during later heads' gaps) instead of being emitted here.
        The c-half normalize is emitted right after its last contributing
        PV, so out-projection of that half can start before the head
        finishes."""
        jmax = (NI // P) * g + (NI // P)  # j-blocks 0..jmax-1 (8 or 16)
        hc, hb = h // 2, h % 2
        e0 = hb * DH  # partition base of this head's 64 dims

        def _ct(jb):
            t = jb - (NI // P) * g
            return 0 if t < 4 else 1

        contrib = [[jb for jb in range(jmax) if _ct(jb) <= c]
                   for c in range(2)]
        done_q = set()
        zps = [ps_z.tile([2 * DH, 512], F32, tag="z", name=f"zps{c}")
               for c in range(2)]

        def norm(c, part=slice(0, 512)):
            # normalize: zT = z * (1/l); rows DH..2DH of zps hold l
            w = part.stop - part.start
            bcr = small.tile([DH, w], F32, tag="bcr")
            nc.vector.reciprocal(bcr, zps[c][DH:2 * DH, part])
            icol = g * NI + c * 512 + part.start
            nc.vector.tensor_mul(
                out=zT_sb[e0:e0 + DH, hc, icol:icol + w],
                in0=zps[c][0:DH, part],
                in1=bcr,
            )

        def pv(jb, pt):
            t = jb - (NI // P) * g
            zlo = max(0, t) * P
            for c in range(_ct(jb), 2):
                c0 = c * 512
                lo = max(zlo, c0)  # masked cols are simply never read
                nc.tensor.matmul(
                    zps[c][:, lo - c0:512],
                    lhsT=v_g[jb // (NSB // NG)][:, jb % (NSB // NG), h, :],
                    rhs=pt[:, lo:c0 + 512],
                    start=(jb == contrib[c][0]),
                    stop=(jb == contrib[c][-1]),
                )
            for c in range(2):
                if jb == contrib[c][-1]:
                    norm(c)

        pending = []
        for jb in range(jmax):
            yield
            t = jb - (NI // P) * g  # >=0 on diagonal strips
            ct = _ct(jb)
            sps = ps_s.tile([P, NI], F32, tag="s")
            pt = ptpool.tile([P, NI], F16, tag="pt")
            zlo = max(0, t) * P
            kap = kT8[e0:e0 + DH, hc, jb, :].rearrange(
                "p (t c) -> p t c", t=2)
            for c in range(ct, 2):
                c0 = c * 512
                lo = max(zlo, c0)
                nc.tensor.matmul(
                    sps[:, lo:c0 + 512],
                    lhsT=kap,
                    rhs=_stride0_pair(
                        qT8[e0:e0 + DH, hc, g * NI + lo:g * NI + c0 + 512],
                        c0 + 512 - lo),
                    start=True,
                    stop=True,
                    perf_mode=PM.DoubleRow,
                )
            nc.scalar.activation(out=pt[:, zlo:NI],
                                 in_=sps[:, zlo:NI], func=AF.Exp)
            if t >= 0:
                # triangle mask on the diagonal 128 columns (GpSimd:
                # SBUF-only op, keeps DVE free)
                nc.gpsimd.tensor_mul(
                    out=pt[:, zlo:zlo + P],
                    in0=pt[:, zlo:zlo + P],
                    in1=masks_sb,
                )
            pending.append((jb, pt))
            if len(pending) > lag:
                pv(*pending.pop(0))
        if sink is not None:
            for item in pending:
                sink(600, (lambda it=item: pv(*it)))
        else:
            for item in pending:
                yield
                pv(*item)

    def attn_head0_split(h, sink):
        """First head (group 0): scores/exp run per 512-col chunk in two
        passes, so Act starts as soon as kq chunk 0 lands instead of
        waiting for chunk 1.  PVs all defer to the sink."""
        g = 0
        jmax = NI // P
        hc, hb = h // 2, h % 2
        e0 = hb * DH
        contrib = [[jb for jb in range(jmax) if jb - (NI // P) * g < 4
                    or c == 1] for c in range(2)]
        zps = [ps_z.tile([2 * DH, 512], F32, tag="z", name=f"zps{c}")
               for c in range(2)]

        def norm(c):
            bcr = small.tile([DH, 512], F32, tag="bcr")
            nc.vector.reciprocal(bcr, zps[c][DH:2 * DH, :])
            icol = c * 512
            nc.vector.tensor_mul(
                out=zT_sb[e0:e0 + DH, hc, icol:icol + 512],
                in0=zps[c][0:DH, :],
                in1=bcr,
            )

        def pv(jb, pt):
            zlo = jb * P
            for c in range(0 if jb < 4 else 1, 2):
                c0 = c * 512
                lo = max(zlo, c0)
                nc.tensor.matmul(
                    zps[c][:, lo - c0:512],
                    lhsT=v_g[0][:, jb, h, :],
                    rhs=pt[:, lo:c0 + 512],
                    start=(jb == contrib[c][0]),
                    stop=(jb == contrib[c][-1]),
                )
            for c in range(2):
                if jb == contrib[c][-1]:
                    norm(c)

        pts = {}
        # pass A: chunk c0 for strips 0-3 (only needs kq chunk 0)
        for jb in range(4):
            yield
            zlo = jb * P
            sps = ps_s.tile([P, NI], F32, tag="s")
            pt = ptpool.tile([P, NI], F16, tag="pt")
            pts[jb] = pt
            kap = kT8[e0:e0 + DH, hc, jb, :].rearrange("p (t c) -> p t c", t=2)
            nc.tensor.matmul(
                sps[:, zlo:512], lhsT=kap,
                rhs=_stride0_pair(qT8[e0:e0 + DH, hc, zlo:512], 512 - zlo),
                start=True, stop=True, perf_mode=PM.DoubleRow)
            nc.scalar.activation(out=pt[:, zlo:512], in_=sps[:, zlo:512],
                                 func=AF.Exp)
            nc.gpsimd.tensor_mul(out=pt[:, zlo:zlo + P],
                                 in0=pt[:, zlo:zlo + P], in1=masks_sb)
        # chunk 0's m=1 parts fill the PE-idle window while xq1 streams in
        # (their data is long since loaded); k's psum->fp8 rides Act, which
        # is idle between pass A and pass B
        emit_kq_part(0, "k", 1, on_act=True)
        emit_kq_part(0, "q", 1)
        # q chunk 1 projects here: pass B's exps need it immediately
        emit_kq_part(1, "q", 0)
        emit_kq_part(1, "q", 1)
        # pass B: chunk c1 for all strips (needs kq chunk 1)
        for jb in range(jmax):
            yield
            zlo = jb * P
            lo = max(zlo, 512)
            sps = ps_s.tile([P, NI], F32, tag="s")
            if jb < 4:
                pt = pts[jb]
            else:
                pt = ptpool.tile([P, NI], F16, tag="pt")
                pts[jb] = pt
            kap = kT8[e0:e0 + DH, hc, jb, :].rearrange("p (t c) -> p t c", t=2)
            nc.tensor.matmul(
                sps[:, lo:NI], lhsT=kap,
                rhs=_stride0_pair(qT8[e0:e0 + DH, hc, lo:NI], NI - lo),
                start=True, stop=True, perf_mode=PM.DoubleRow)
            nc.scalar.activation(out=pt[:, lo:NI], in_=sps[:, lo:NI],
                                 func=AF.Exp)
            if jb >= 4:
                nc.gpsimd.tensor_mul(out=pt[:, zlo:zlo + P],
                                     in0=pt[:, zlo:zlo + P], in1=masks_sb)
        for jb in range(jmax):
            sink(600, (lambda it=(jb, pts[jb]): pv(*it)))

    def outproj_half(ib, d2, box, late=False):
        # output projection for one d-half of one 128-row i-block -> fp16
        # SBUF; the d2=1 half DMAs the block to HBM.  Split per half so a
        # FIFO pop overshoots small strip gaps by half as much.  For the
        # tail blocks (late=True) the second copy runs on Act, which is
        # idle once the exps are done, halving the tail chain.
        if d2 == 0:
            box["osb"] = outpool.tile([P, D], F16, tag="o", name="osb")
        osb = box["osb"]
        if True:
            # late blocks borrow the (idle) score-strip psum pool for the
            # second half so the copy pipeline runs 4 deep instead of 2
            if late and d2 == 1:
                ops_full = ps_s.tile([P, NI], F32, tag="s", name="opsl")
                ops = ops_full[:, 0:512]
            else:
                ops = ps_mm.tile([P, 512], F32, tag="mm")
            for ec in range(EC):
                nc.tensor.matmul(
                    ops,
                    lhsT=zT_sb[:, ec, ib * P:(ib + 1) * P],
                    rhs=wo_sb[:, ec, d2 * 512:(d2 + 1) * 512],
                    start=(ec == 0),
                    stop=(ec == EC - 1),
                )
            if late and d2 == 1:
                nc.scalar.activation(
                    out=osb[:, d2 * 512:(d2 + 1) * 512], in_=ops,
                    func=AF.Copy)
            else:
                nc.vector.tensor_copy(
                    out=osb[:, d2 * 512:(d2 + 1) * 512], in_=ops)
        if d2 == 1:
            eng = nc.gpsimd if ib % 2 == 0 else nc.sync
            eng.dma_start(out=out[ib * P:(ib + 1) * P, :], in_=osb)

    def outproj_block(ib, late=False):
        box = {}
        outproj_half(ib, 0, box, late)
        outproj_half(ib, 1, box, late)

    def push_block(ib, late=False):
        box = {}
        for d2 in range(2):
            push(1050, lambda d2=d2: outproj_half(ib, d2, box, late))

    # ---- emission schedule ----
    # Drive each attention head as a generator, feeding one filler item
    # (projection chunk / outproj block) per strip gap so the PE never
    # starves while Act grinds exp.
    def drive(gen, fillers):
        for _ in gen:
            if fillers:
                fillers.pop(0)()

    emit_init()
    # prefetch all inputs in consumption-priority order (transfers serialize
    # on the shared DMA device, so queue order == arrival order).  The k/q
    # path gates the first exp, so it loads first; v arrivals are absorbed
    # by the lagged PV pipeline.
    load_kq_chunk(0, "k")
    load_kq_chunk(0, "q")
    load_kq_w("k")
    load_kq_w("q")
    nc.scalar.dma_start(out=masks_sb, in_=masks)
    # q chunk 1 before k chunk 1: the first head's pass-B exps need q cols
    # 512-1023 but only k blocks 0-3
    load_kq_chunk(1, "q")
    load_kq_chunk(1, "k")
    load_v_piece(0)
    load_v_piece(1)
    load_kq_chunk(2, "k")
    load_kq_chunk(2, "q")
    load_kq_chunk(3, "k")
    load_kq_chunk(3, "q")
    for pc in range(2, 8):
        load_v_piece(pc)
    nc.sync.dma_start(out=wo_sb,
                      in_=ten["wo"].rearrange("(c p) d -> p c d", p=P))

    # only the m=0 parts of chunk 0 project upfront (all pass-A needs);
    # the m=1 parts fill the PE-idle xq1 DMA wait inside the first head
    emit_kq_part(0, "k", 0)
    emit_kq_part(0, "q", 0)

    # ---- global deferred-work FIFO ----
    # All PE work that is not score/exp-critical (v projections, late kq
    # chunks, deferred PVs, out-projections) flows through one FIFO in
    # dependency order and is pumped during strip gaps at a cycle budget
    # matched to Act's exp pace.  This keeps Act saturated while the PE
    # never sits blocked at the head of its stream.
    from collections import deque

    fifo = deque()

    def push(cost, fn):
        fifo.append((cost, fn))

    def pump(budget):
        while fifo and budget > 0:
            cost, fn = fifo.popleft()
            fn()
            budget -= cost

    # static items, ordered by their input-DMA arrival.  k chunk 1 leads
    # (the first head's pass-B strips 4-7 need those k blocks).
    for m in range(EC):
        push(2100, lambda m=m: emit_kq_part(1, "k", m))
    for sb in range(4):
        push(2100, lambda sb=sb: emit_v_block(sb))
    for n in (2, 3):
        for which in ("k", "q"):
            for m in range(EC):
                push(2100, lambda n=n, w=which, m=m: emit_kq_part(n, w, m))
    for sb in range(4, 8):
        push(2100, lambda sb=sb: emit_v_block(sb))

    def run_head(g, h, lag, budgets=None):
        jmax = (NI // P) * g + (NI // P)
        for i, _ in enumerate(attn_head(g, h, lag=lag, sink=push)):
            if budgets is not None:
                b = budgets[i] if i < len(budgets) else budgets[-1]
            else:
                # pump budget ~= this gap's Act slack: exp width minus the
                # strip's own scores and sync overheads
                w = NI - max(0, i - (NI // P) * g) * P if i < jmax else 512
                b = max(0, 2 * w - 600)
            # never pump ahead of the first strips: at a head transition Act
            # has no buffered exp work, so the new head's scores must lead
            if i == 0:
                b = 0
            elif i == 1:
                b = min(b, 600)
            pump(b)

    # three g0 heads first: their exps need only kq chunks 0/1, giving Act
    # cover while the rest of the inputs stream in.  The first head's exps
    # run per 512-chunk (pass A needs only kq chunk 0).  Early gaps don't
    # pump (nothing has arrived yet).
    h0_budgets = [0, 0, 0, 0, 2200, 2200, 0, 0, 0, 0, 600, 600]
    for i, _ in enumerate(attn_head0_split(0, sink=push)):
        pump(h0_budgets[i] if i < len(h0_budgets) else 600)
    run_head(0, 1, 8)
    run_head(0, 2, 8)
    for sb in range(8, 16):
        push(2100, lambda sb=sb: emit_v_block(sb))
    run_head(1, 0, 8)
    run_head(0, 3, 8)
    # outproj of group 0: depends on g0.h3's norms, which were just pushed
    for ib in range(NI // P):
        push_block(ib)
    run_head(1, 1, 8)
    run_head(1, 2, 8)
    # g1.h3 runs inline (last head, lag-4) with quarter-granular norms, so
    # its out-projection blocks chase the PV pipeline instead of queueing
    # in the tail.  Quarter (c,q) completes with PV(s{9,11,13,15}).
    def push_op(ib):
        push_block(ib, late=True)

    # pre-drain g1.h2's deferred PVs/norms so g1.h3's inline PVs (which
    # need the zps slots they release) can't deadlock against low budgets
    pump(8000)
    g3 = attn_head(1, 3, lag=4)
    for i in range(16):
        next(g3)
        pump(0 if i == 0 else max(0, 2 * (NI - max(0, i - 8) * P) - 600))
    # drain the remaining PVs FIRST so the final c1-norm lands as early as
    # possible; only then pump the out-projection blocks (which pipeline
    # 4-deep through ps_mm+ps_s and never gate anything upstream)
    for _ in g3:  # pv(s12..15) + final norms
        pass
    for ib in range(NI // P, 2 * (NI // P)):
        push_op(ib)
    pump(1 << 30)


def build_nc():
    from contextlib import ExitStack

    nc = bass.Bass()
    ten = {}
    ten["xq"] = nc.dram_tensor("xq", [D, S], F8, kind="ExternalInput")[:]
    ten["xk"] = nc.dram_tensor("xk", [D, S], F8, kind="ExternalInput")[:]
    ten["xv"] = nc.dram_tensor("xv", [D, S], F16, kind="ExternalInput")[:]
    for name in ("wqhi", "wqlo", "wkhi", "wklo"):
        ten[name] = nc.dram_tensor(name, [P, DC * E], F8,
                                   kind="ExternalInput")[:]
    ten["wv"] = nc.dram_tensor("wv", [D, E], F16, kind="ExternalInput")[:]
    ten["wo"] = nc.dram_tensor("wo", [E, D], F16, kind="ExternalInput")[:]
    ten["bq"] = nc.dram_tensor("bq", [P, EC], F32, kind="ExternalInput")[:]
    ten["bk"] = nc.dram_tensor("bk", [P, EC], F32, kind="ExternalInput")[:]
    ten["bv"] = nc.dram_tensor("bv", [E], F32, kind="ExternalInput")[:]
    ten["masks"] = nc.dram_tensor("masks", [P, P], F16, kind="ExternalInput")[:]
    ten["out"] = nc.dram_tensor("out", [S, D], F16, kind="ExternalOutput")[:]
    with tile.TileContext(nc) as tc:
        with ExitStack() as ctx:
            _emit(ctx, tc, ten)
    return nc


_CACHE = {}


def _get_nc():
    if "nc" not in _CACHE:
        _CACHE["nc"] = build_nc()
    return _CACHE["nc"]


def _pack_w8(W):
    """[D, E] fp32 -> (hi, lo) e4m3, each [128, DC2*2*E] with
    w8[p, dc2, t, e] = W[dc2*256 + t*128 + p, e]."""
    import ml_dtypes
    e4 = ml_dtypes.float8_e4m3fn
    Wf = np.asarray(W, dtype=np.float32)
    hi = Wf.astype(e4)
    lo = (Wf - hi.astype(np.float32)).astype(e4)
    outs = []
    for w8 in (hi, lo):
        r = w8.reshape(DC2, 2, P, E).transpose(2, 0, 1, 3).reshape(P, -1)
        outs.append(np.ascontiguousarray(r))
    return outs


def make_in_maps(query_input, key_input, value_input, W_Q, W_K, W_V, W_O,
                 b_Q, b_K, b_V, b_O):
    import ml_dtypes
    e4 = ml_dtypes.float8_e4m3fn

    qi = np.asarray(query_input, dtype=np.float32)
    ki = np.asarray(key_input, dtype=np.float32)
    vi = np.asarray(value_input, dtype=np.float32)
    W_Q = np.asarray(W_Q, dtype=np.float32)
    W_K = np.asarray(W_K, dtype=np.float32)
    W_V = np.asarray(W_V, dtype=np.float32)
    W_O = np.asarray(W_O, dtype=np.float32)
    b_Q = np.asarray(b_Q, dtype=np.float32)
    b_K = np.asarray(b_K, dtype=np.float32)
    b_V = np.asarray(b_V, dtype=np.float32)

    masks = np.triu(np.ones((P, P), dtype=np.float16))  # tri[j, i] = i >= j

    xT8, xT16 = {}, {}
    for b in range(B):
        xT8[("q", b)] = np.ascontiguousarray(qi[b].T).astype(e4)
        xT8[("k", b)] = np.ascontiguousarray(ki[b].T).astype(e4)
        xT16[("v", b)] = np.ascontiguousarray(vi[b].T).astype(np.float16)

    in_maps = []
    for core in range(NCORES):
        b, hg = core // (NCORES // B), core % (NCORES // B)
        hs = slice(hg * HL, (hg + 1) * HL)
        wq = np.transpose(W_Q[hs], (1, 0, 2)).reshape(D, E)
        wk = np.transpose(W_K[hs], (1, 0, 2)).reshape(D, E)
        wqhi, wqlo = _pack_w8(wq)
        wkhi, wklo = _pack_w8(wk)
        in_maps.append({
            "xq": xT8[("q", b)],
            "xk": xT8[("k", b)],
            "xv": xT16[("v", b)],
            "wqhi": wqhi, "wqlo": wqlo,
            "wkhi": wkhi, "wklo": wklo,
            "wv": np.ascontiguousarray(
                np.transpose(W_V[hs], (1, 0, 2)).reshape(D, E)).astype(np.float16),
            "wo": np.ascontiguousarray(
                W_O[hs].reshape(E, D)).astype(np.float16),
            "bq": np.ascontiguousarray(
                (b_Q[hs].reshape(EC, P).T / ATTN_SCALE).astype(np.float32)),
            "bk": np.ascontiguousarray(
                b_K[hs].reshape(EC, P).T.astype(np.float32)),
            "bv": np.ascontiguousarray(b_V[hs].reshape(E)),
            "masks": masks,
        })
    return in_maps


def gather_out(results, b_O):
    out = np.zeros((B, S, D), dtype=np.float64)
    for core in range(NCORES):
        out[core // (NCORES // B)] += results[core]["out"].astype(np.float64)
    out += np.asarray(b_O, dtype=np.float64)
    return out.astype(np.float32)


def kernel(query_input, key_input, value_input, W_Q, W_K, W_V, W_O,
           b_Q, b_K, b_V, b_O):
    nc = _get_nc()
    in_maps = make_in_maps(query_input, key_input, value_input,
                           W_Q, W_K, W_V, W_O, b_Q, b_K, b_V, b_O)
    res = run_bass_kernel_spmd(nc, in_maps, list(range(NCORES)))
    return gather_out(res.results, b_O)


def kernel_timed(inputs, trace_cores=None, **kwargs):
    """Like kernel() but traces and returns (out, BassKernelResults)."""
    nc = _get_nc()
    in_maps = make_in_maps(**inputs)
    res = run_bass_kernel_spmd(
        nc, in_maps, list(range(NCORES)), trace=True,
        trace_cores=trace_cores, **kwargs)
    return gather_out(res.results, inputs["b_O"]), res
